# revision 22
# baseline (speedup 1.0000x reference)
"""CKConv GNN message-passing kernel for 8 Trainium2 NeuronCores.

Strategy: shard edges E=30000 across 8 cores (3750 each, padded to 3840).
Host gathers node rows / relative times per edge shard; each core runs the
two SIREN kernel-nets and the per-edge matvec fused:

  msg[e,h] = sum_{j,k} h2[e,j] * emb[e,k] * w3[j, h*64+k]

computed as a per-edge outer product P[e,(j,k)] (DVE), DMA-transposed to
put (j,k) on partitions, then one PSUM-accumulated matmul chain against the
host-permuted weight w3p[(j,k), h].  Outputs per-edge messages (transposed);
host scatter-adds into the node accumulators (segment sum).
"""
import sys

sys.path.insert(0, "/opt/trn_rl_repo")

import numpy as np

H = 64
KH = 50
OMEGA = 30.0
NCORES = 8
E_TOT = 30000
EPC = E_TOT // NCORES      # 3750 edges per core
NT = 30                    # 128-edge tiles per core (30*128 = 3840 padded)
EPT = NT * 128             # 3840
NU, NI = 10000, 20000
TWO_PI = 2.0 * np.pi
C_RED = OMEGA / TWO_PI     # omega/2pi for sin range reduction


def _build_nc(trivial_u, trivial_i, repeat=1, parts=("pbuild", "ptdma", "mm", "h2dup")):
    import contextlib
    import concourse.bass as bass
    import concourse.tile as tile
    from concourse import bacc, mybir
    from concourse.masks import make_identity
    from contextlib import ExitStack

    f32 = mybir.dt.float32
    f16 = mybir.dt.float16
    i32 = mybir.dt.int32
    Alu = mybir.AluOpType
    Act = mybir.ActivationFunctionType

    nc = bacc.Bacc("TRN2")

    # ---- DRAM tensors (per-core inputs) ----
    dram = {}
    for net in ("u", "i"):
        dram[f"rel_{net}"] = nc.dram_tensor(f"rel_{net}", [128, NT], f32, kind="ExternalInput")
        dram[f"emb_{net}"] = nc.dram_tensor(f"emb_{net}", [128, NT, H], f16, kind="ExternalInput")
        dram[f"w1b_{net}"] = nc.dram_tensor(f"w1b_{net}", [128, KH], f32, kind="ExternalInput")
        dram[f"w2_{net}"] = nc.dram_tensor(f"w2_{net}", [KH, KH], f32, kind="ExternalInput")
        dram[f"w3p_{net}"] = nc.dram_tensor(f"w3p_{net}", [128, 25, H], f16, kind="ExternalInput")
        dram[f"msgT_{net}"] = nc.dram_tensor(f"msgT_{net}", [H, EPT], f32, kind="ExternalOutput")

    def bview(ap, newdims):
        """Make an AP view with explicit [step, count] free dims appended."""
        return bass.AP(tensor=ap.tensor, offset=ap.offset, ap=newdims)

    with tile.TileContext(nc) as tc:
        with ExitStack() as ctx:
            singles = ctx.enter_context(tc.tile_pool(name="singles", bufs=1))
            stage = ctx.enter_context(tc.tile_pool(name="stage", bufs=1))
            ptiles = ctx.enter_context(tc.tile_pool(name="ptiles", bufs=2))
            scratch = ctx.enter_context(tc.tile_pool(name="scratch", bufs=2))
            small = ctx.enter_context(tc.tile_pool(name="small", bufs=4))
            psum_b = ctx.enter_context(tc.tile_pool(name="psum_b", bufs=1, space="PSUM"))
            psum_s = ctx.enter_context(tc.tile_pool(name="psum_s", bufs=2, space="PSUM"))

            ident = singles.tile([128, 128], f32)
            make_identity(nc, ident[:])
            eps = singles.tile([128, 1], f32)
            nc.vector.memset(eps[:], 1e-5)

            loop_cm = tc.For_i(0, repeat, 1) if repeat > 1 else contextlib.nullcontext()
            with loop_cm:
              for net, trivial in (("u", trivial_u), ("i", trivial_i)):
                w1b = singles.tile([128, KH], f32, tag=f"w1b_{net}")
                nc.gpsimd.dma_start(out=w1b[:], in_=dram[f"w1b_{net}"][:, :])
                w2s = singles.tile([KH, KH], f32, tag=f"w2_{net}")
                nc.gpsimd.dma_start(out=w2s[:], in_=dram[f"w2_{net}"][:, :])
                w3ps = singles.tile([128, 25, H], f16, tag=f"w3p_{net}")
                nc.gpsimd.dma_start(out=w3ps[:], in_=dram[f"w3p_{net}"][:, :, :])
                relS = stage.tile([128, NT], f32, tag=f"rel_{net}")
                nc.gpsimd.dma_start(out=relS[:], in_=dram[f"rel_{net}"][:, :])
                embS = stage.tile([128, NT, H], f16, tag=f"emb_{net}")
                nc.gpsimd.dma_start(out=embS[:], in_=dram[f"emb_{net}"][:, :, :])

                # ---------- stage A: h1pre[e, t, f] = rel[e,t] * w1[f] ----------
                h1pre = scratch.tile([128, NT, KH], f32, tag="h1pre")
                rel_v = bview(relS[:], [relS[:].ap[0], [1, NT], [0, KH]])
                w1_v = bview(w1b[:], [w1b[:].ap[0], [0, NT], [1, KH]])
                nc.vector.tensor_tensor(out=h1pre[:], in0=rel_v, in1=w1_v, op=Alu.mult)

                def layernorm_sin(x_ap, out_tile, tag):
                    """x_ap: [128, NT, KH] (psum or sbuf). Writes sin(OMEGA*LN(x)) to out_tile (f32/f16)."""
                    # mean & var via reduces
                    sums = small.tile([128, NT], f32, tag="sums")
                    nc.vector.tensor_reduce(out=sums[:], in_=x_ap, axis=mybir.AxisListType.X, op=Alu.add)
                    sq = scratch.tile([128, NT, KH], f32, tag="sq")
                    nc.scalar.square(out=sq[:], in_=x_ap)
                    sums2 = small.tile([128, NT], f32, tag="sums2")
                    nc.vector.tensor_reduce(out=sums2[:], in_=sq[:], axis=mybir.AxisListType.X, op=Alu.add)
                    mean = small.tile([128, NT], f32, tag="mean")
                    nc.vector.tensor_scalar(out=mean[:], in0=sums[:], scalar1=1.0 / KH,
                                            scalar2=None, op0=Alu.mult)
                    # var = sums2/KH - mean^2   (biased, matches jnp.var)
                    m2 = small.tile([128, NT], f32, tag="m2")
                    nc.vector.tensor_tensor(out=m2[:], in0=mean[:], in1=mean[:], op=Alu.mult)
                    var = small.tile([128, NT], f32, tag="var")
                    nc.vector.tensor_scalar(out=var[:], in0=sums2[:], scalar1=1.0 / KH,
                                            scalar2=None, op0=Alu.mult)
                    nc.vector.tensor_tensor(out=var[:], in0=var[:], in1=m2[:], op=Alu.subtract)
                    # rstd = 1/sqrt(var + 1e-5), then fold in C_RED for range reduction
                    sd = small.tile([128, NT], f32, tag="sd")
                    nc.scalar.activation(out=sd[:], in_=var[:], func=Act.Sqrt,
                                         bias=eps[:], scale=1.0)
                    rstd = small.tile([128, NT], f32, tag="rstd")
                    nc.vector.reciprocal(out=rstd[:], in_=sd[:])
                    rstdc = small.tile([128, NT], f32, tag="rstdc")
                    nc.vector.tensor_scalar(out=rstdc[:], in0=rstd[:], scalar1=C_RED,
                                            scalar2=None, op0=Alu.mult)
                    # y = (x - mean) * rstdc   -> sin arg / 2pi
                    y = scratch.tile([128, NT, KH], f32, tag="y")
                    mean_v = bview(mean[:], [mean[:].ap[0], [1, NT], [0, KH]])
                    rstdc_v = bview(rstdc[:], [rstdc[:].ap[0], [1, NT], [0, KH]])
                    nc.vector.tensor_tensor(out=y[:], in0=x_ap, in1=mean_v, op=Alu.subtract)
                    nc.vector.tensor_tensor(out=y[:], in0=y[:], in1=rstdc_v, op=Alu.mult)
                    # f = y - round(y) via int32 round-cast
                    yi = scratch.tile([128, NT, KH], i32, tag="yi")
                    nc.vector.tensor_copy(out=yi[:], in_=y[:])
                    yif = scratch.tile([128, NT, KH], f32, tag="yif")
                    nc.vector.tensor_copy(out=yif[:], in_=yi[:])
                    nc.vector.tensor_tensor(out=y[:], in0=y[:], in1=yif[:], op=Alu.subtract)
                    # out = sin(2pi * f)
                    nc.scalar.activation(out=out_tile[:], in_=y[:], func=Act.Sin,
                                         scale=TWO_PI, bias=0.0)

                h1S = stage.tile([128, NT, KH], f32, tag=f"h1S_{net}")
                layernorm_sin(h1pre[:], h1S, f"l1{net}")

                # ---------- stage B: h2pre[:, t, :] = h1[:, t, :] @ w2 ----------
                h2pre_ps = psum_b.tile([128, NT, 64], f32, tag="h2pre", space="PSUM")
                for t in range(NT):
                    h1T_ps = psum_s.tile([KH, 128], f32, tag="h1T", space="PSUM")
                    nc.tensor.transpose(out=h1T_ps[:], in_=h1S[:, t, :], identity=ident[:])
                    h1T = small.tile([KH, 128], f32, tag="h1T_sb")
                    nc.scalar.activation(out=h1T[:], in_=h1T_ps[:], func=Act.Copy,
                                         bias=0.0, scale=1.0)
                    nc.tensor.matmul(out=h2pre_ps[:, t, 0:KH], lhsT=h1T[:], rhs=w2s[:, :],
                                     start=True, stop=True)

                h2pre_v = h2pre_ps[:, :, 0:KH]
                h2S = stage.tile([128, NT, KH], f16, tag=f"h2S_{net}")
                layernorm_sin(h2pre_v, h2S, f"l2{net}")

                # duplicate each h2 value (pairs along free dim) so the P-build
                # tensor_tensor reads aligned 16-bit pairs -> 2x DVE mode
                h2dupS = None
                if "h2dup" in parts:
                    h2dupS = stage.tile([128, NT, KH, 2], f16, tag=f"h2dup_{net}")
                    h2b = h2S[:]
                    h2_dup_view = bview(h2b, [h2b.ap[0], [KH, NT], [1, KH], [0, 2]])
                    nc.vector.tensor_copy(out=h2dupS[:], in_=h2_dup_view)

                # ---------- per-tile: P build, transpose, matmul chain ----------
                msgT_d = dram[f"msgT_{net}"]
                TGRP = 3  # e-tiles per DMA-transpose instruction
                for tp in range(0, NT, TGRP):
                    P = ptiles.tile([128, TGRP, KH, H], f16, tag="P")
                    PT = ptiles.tile([128, TGRP * 25, 128], f16, tag="PT")
                    if "pbuild" in parts:
                        for s in range(TGRP):
                            t = tp + s
                            emb_t = embS[:, t, :]
                            Ps = P[:, s, :, :]
                            if h2dupS is not None:
                                hd = h2dupS[:, t, :, :]
                                h2_v = bview(hd, [hd.ap[0], [2, KH], [0, 32], [1, 2]])
                                emb_v = bview(emb_t, [emb_t.ap[0], [0, KH], [2, 32], [1, 2]])
                                P_v = bview(Ps, [Ps.ap[0], [H, KH], [2, 32], [1, 2]])
                                nc.vector.tensor_tensor(out=P_v, in0=h2_v, in1=emb_v, op=Alu.mult)
                            else:
                                h2_t = h2S[:, t, :]
                                h2_v = bview(h2_t, [h2_t.ap[0], [1, KH], [0, H]])
                                emb_v = bview(emb_t, [emb_t.ap[0], [0, KH], [1, H]])
                                nc.vector.tensor_tensor(out=Ps, in0=h2_v, in1=emb_v, op=Alu.mult)

                    if "ptdma" in parts:
                        nc.sync.dma_start_transpose(
                            out=PT[:], in_=P[:].rearrange("p s j k -> p (s j k)"))

                    if "mm" in parts:
                        for s in range(TGRP):
                            t = tp + s
                            msg_ps = psum_s.tile([H, 128], f32, tag="msg", space="PSUM")
                            for c in range(25):
                                nc.tensor.matmul(out=msg_ps[:], lhsT=w3ps[:, c, :],
                                                 rhs=PT[:, s * 25 + c, :],
                                                 start=(c == 0), stop=(c == 24))
                            msg_sb = small.tile([H, 128], f32, tag="msg_sb")
                            nc.scalar.activation(out=msg_sb[:], in_=msg_ps[:], func=Act.Copy,
                                                 bias=0.0, scale=1.0)
                            meng = nc.gpsimd if "gmsg" in parts else nc.sync
                            meng.dma_start(out=msgT_d[:, t * 128:(t + 1) * 128], in_=msg_sb[:])

    nc.compile()
    return nc


_NC_CACHE = {}


def _get_nc(trivial_u, trivial_i, repeat=1, parts=("pbuild", "ptdma", "mm", "h2dup")):
    key = (trivial_u, trivial_i, repeat, tuple(parts))
    if key not in _NC_CACHE:
        _NC_CACHE[key] = _build_nc(trivial_u, trivial_i, repeat, parts)
    return _NC_CACHE[key]


def _trivial(p):
    names = ["w1", "b1", "g1", "be1", "w2", "b2", "g2", "be2", "w3", "b3"]
    d = dict(zip(names, p))
    return (np.all(d["b1"] == 0) and np.all(d["g1"] == 1) and np.all(d["be1"] == 0)
            and np.all(d["b2"] == 0) and np.all(d["g2"] == 1) and np.all(d["be2"] == 0)
            and np.all(d["b3"] == 0))


def prepare_in_maps(u_embedded, i_embedded, user_per_trans, item_per_trans, edges_t, u_t, i_t,
                    uw1, ub1, ug1, ube1, uw2, ub2, ug2, ube2, uw3, ub3,
                    iw1, ib1, ig1, ibe1, iw2, ib2, ig2, ibe2, iw3, ib3):
    u_embedded = np.asarray(u_embedded, np.float32)
    i_embedded = np.asarray(i_embedded, np.float32)
    user_idx = np.asarray(user_per_trans)
    item_idx = np.asarray(item_per_trans)
    edges_t = np.asarray(edges_t, np.float32)
    u_t = np.asarray(u_t, np.float32)
    i_t = np.asarray(i_t, np.float32)

    up = [np.asarray(x, np.float32) for x in (uw1, ub1, ug1, ube1, uw2, ub2, ug2, ube2, uw3, ub3)]
    ip = [np.asarray(x, np.float32) for x in (iw1, ib1, ig1, ibe1, iw2, ib2, ig2, ibe2, iw3, ib3)]
    triv_u, triv_i = _trivial(up), _trivial(ip)
    if not (triv_u and triv_i):
        raise NotImplementedError("non-trivial LN affine / bias params not supported")

    # host gather
    rel_u = (u_t[user_idx] - edges_t).astype(np.float32)       # [E]
    rel_i = (i_t[item_idx] - edges_t).astype(np.float32)
    gu = u_embedded[user_idx].astype(np.float16)               # [E, H]
    gi = i_embedded[item_idx].astype(np.float16)

    def w3perm(w3):
        # w3p[(j,k), h] = w3[j, h*64+k]; chunked then partition-major [128, 25, 64]
        w3r = w3.reshape(KH, H, H)                              # [j, h, k]
        w3p = np.ascontiguousarray(w3r.transpose(0, 2, 1)).reshape(25, 128, H)
        return np.ascontiguousarray(w3p.transpose(1, 0, 2)).astype(np.float16)

    consts = {}
    for net, p in (("u", up), ("i", ip)):
        consts[f"w1b_{net}"] = np.ascontiguousarray(np.tile(p[0][None, :], (128, 1)))
        consts[f"w2_{net}"] = np.ascontiguousarray(p[4])
        consts[f"w3p_{net}"] = w3perm(p[8])

    def shard(a, m):
        # pad core shard to EPT rows, then tile-major -> partition-major
        sl = a[m * EPC:(m + 1) * EPC]
        out = np.zeros((EPT,) + a.shape[1:], a.dtype)
        out[:EPC] = sl
        out = out.reshape((NT, 128) + a.shape[1:])
        return np.ascontiguousarray(np.moveaxis(out, 1, 0))    # [128, NT, ...]

    in_maps = []
    for m in range(NCORES):
        im = {
            "rel_u": shard(rel_u, m),
            "rel_i": shard(rel_i, m),
            "emb_u": shard(gu, m),
            "emb_i": shard(gi, m),
        }
        im.update(consts)
        in_maps.append(im)
    return in_maps, (triv_u, triv_i), (user_idx, item_idx)


def kernel(**inputs):
    from concourse.bass_utils import run_bass_kernel_spmd
    import os
    in_maps, (triv_u, triv_i), (user_idx, item_idx) = prepare_in_maps(**inputs)
    parts = tuple(os.environ.get("CK_PARTS", "pbuild,ptdma,mm,h2dup,gmsg").split(","))
    nc = _get_nc(triv_u, triv_i, 1, parts)
    trace = bool(os.environ.get("CK_TRACE"))
    res = run_bass_kernel_spmd(nc, in_maps, core_ids=list(range(NCORES)), trace=trace)
    global LAST_RESULT
    LAST_RESULT = res

    msg_u = np.concatenate([res.results[m]["msgT_u"].T[:EPC] for m in range(NCORES)], axis=0)
    msg_i = np.concatenate([res.results[m]["msgT_i"].T[:EPC] for m in range(NCORES)], axis=0)

    hLu = np.zeros((NU, H), np.float32)
    np.add.at(hLu, user_idx, msg_i)
    hLi = np.zeros((NI, H), np.float32)
    np.add.at(hLi, item_idx, msg_u)
    return (hLu, hLi)


# revision 25
# speedup vs baseline: 4.3451x; 4.3451x over previous
"""CKConv GNN message-passing kernel for 8 Trainium2 NeuronCores.

Strategy: shard edges E=30000 across 8 cores (3750 each, padded to 3840).
Host gathers node rows / relative times per edge shard; each core runs the
two SIREN kernel-nets and the per-edge matvec fused:

  msg[e,h] = sum_{j,k} h2[e,j] * emb[e,k] * w3[j, h*64+k]

computed as a per-edge outer product P[e,(j,k)] (DVE), DMA-transposed to
put (j,k) on partitions, then one PSUM-accumulated matmul chain against the
host-permuted weight w3p[(j,k), h].  Outputs per-edge messages (transposed);
host scatter-adds into the node accumulators (segment sum).
"""
import sys

sys.path.insert(0, "/opt/trn_rl_repo")

import numpy as np

H = 64
KH = 50
OMEGA = 30.0
NCORES = 8
E_TOT = 30000
EPC = E_TOT // NCORES      # 3750 edges per core
NT = 30                    # 128-edge tiles per core (30*128 = 3840 padded)
EPT = NT * 128             # 3840
NU, NI = 10000, 20000
TWO_PI = 2.0 * np.pi
C_RED = OMEGA / TWO_PI     # omega/2pi for sin range reduction


def _build_nc(trivial_u, trivial_i, repeat=1, parts=("pbuild", "ptdma", "mm", "h2dup")):
    import contextlib
    import concourse.bass as bass
    import concourse.tile as tile
    from concourse import bacc, mybir
    from concourse.masks import make_identity
    from contextlib import ExitStack

    f32 = mybir.dt.float32
    f16 = mybir.dt.float16
    i32 = mybir.dt.int32
    Alu = mybir.AluOpType
    Act = mybir.ActivationFunctionType

    nc = bacc.Bacc("TRN2")

    # ---- DRAM tensors (per-core inputs) ----
    dram = {}
    for net in ("u", "i"):
        dram[f"rel_{net}"] = nc.dram_tensor(f"rel_{net}", [128, NT], f32, kind="ExternalInput")
        dram[f"emb_{net}"] = nc.dram_tensor(f"emb_{net}", [128, NT, H], f16, kind="ExternalInput")
        dram[f"w1b_{net}"] = nc.dram_tensor(f"w1b_{net}", [128, KH], f32, kind="ExternalInput")
        dram[f"w2_{net}"] = nc.dram_tensor(f"w2_{net}", [KH, KH], f32, kind="ExternalInput")
        dram[f"w3p_{net}"] = nc.dram_tensor(f"w3p_{net}", [128, 25, H], f16, kind="ExternalInput")
        dram[f"msgT_{net}"] = nc.dram_tensor(f"msgT_{net}", [H, EPT], f32, kind="ExternalOutput")

    def bview(ap, newdims):
        """Make an AP view with explicit [step, count] free dims appended."""
        return bass.AP(tensor=ap.tensor, offset=ap.offset, ap=newdims)

    with tile.TileContext(nc) as tc:
        with ExitStack() as ctx:
            singles = ctx.enter_context(tc.tile_pool(name="singles", bufs=1))
            stage = ctx.enter_context(tc.tile_pool(name="stage", bufs=1))
            ptiles = ctx.enter_context(tc.tile_pool(name="ptiles", bufs=2))
            scratch = ctx.enter_context(tc.tile_pool(name="scratch", bufs=2))
            small = ctx.enter_context(tc.tile_pool(name="small", bufs=4))
            psum_b = ctx.enter_context(tc.tile_pool(name="psum_b", bufs=1, space="PSUM"))
            psum_s = ctx.enter_context(tc.tile_pool(name="psum_s", bufs=2, space="PSUM"))
            psum_m = ctx.enter_context(tc.tile_pool(name="psum_m", bufs=2, space="PSUM"))

            ident = singles.tile([128, 128], f32)
            make_identity(nc, ident[:])
            eps = singles.tile([128, 1], f32)
            nc.vector.memset(eps[:], 1e-5)

            loop_cm = tc.For_i(0, repeat, 1) if repeat > 1 else contextlib.nullcontext()
            with loop_cm:
              for net, trivial in (("u", trivial_u), ("i", trivial_i)):
                w1b = singles.tile([128, KH], f32, tag=f"w1b_{net}")
                nc.gpsimd.dma_start(out=w1b[:], in_=dram[f"w1b_{net}"][:, :])
                w2s = singles.tile([KH, KH], f32, tag=f"w2_{net}")
                nc.gpsimd.dma_start(out=w2s[:], in_=dram[f"w2_{net}"][:, :])
                w3ps = singles.tile([128, 25, H], f16, tag=f"w3p_{net}")
                nc.gpsimd.dma_start(out=w3ps[:], in_=dram[f"w3p_{net}"][:, :, :])
                relS = stage.tile([128, NT], f32, tag=f"rel_{net}")
                nc.gpsimd.dma_start(out=relS[:], in_=dram[f"rel_{net}"][:, :])
                embS = stage.tile([128, NT, H], f16, tag=f"emb_{net}")
                nc.gpsimd.dma_start(out=embS[:], in_=dram[f"emb_{net}"][:, :, :])

                # ---------- stage A: h1pre[e, t, f] = rel[e,t] * w1[f] ----------
                h1pre = scratch.tile([128, NT, KH], f32, tag="h1pre")
                rel_v = bview(relS[:], [relS[:].ap[0], [1, NT], [0, KH]])
                w1_v = bview(w1b[:], [w1b[:].ap[0], [0, NT], [1, KH]])
                nc.vector.tensor_tensor(out=h1pre[:], in0=rel_v, in1=w1_v, op=Alu.mult)

                def layernorm_sin(x_ap, out_tile, tag):
                    """x_ap: [128, NT, KH] (psum or sbuf). Writes sin(OMEGA*LN(x)) to out_tile (f32/f16)."""
                    # mean & var via reduces
                    sums = small.tile([128, NT], f32, tag="sums")
                    nc.vector.tensor_reduce(out=sums[:], in_=x_ap, axis=mybir.AxisListType.X, op=Alu.add)
                    sq = scratch.tile([128, NT, KH], f32, tag="sq")
                    nc.scalar.square(out=sq[:], in_=x_ap)
                    sums2 = small.tile([128, NT], f32, tag="sums2")
                    nc.vector.tensor_reduce(out=sums2[:], in_=sq[:], axis=mybir.AxisListType.X, op=Alu.add)
                    mean = small.tile([128, NT], f32, tag="mean")
                    nc.vector.tensor_scalar(out=mean[:], in0=sums[:], scalar1=1.0 / KH,
                                            scalar2=None, op0=Alu.mult)
                    # var = sums2/KH - mean^2   (biased, matches jnp.var)
                    m2 = small.tile([128, NT], f32, tag="m2")
                    nc.vector.tensor_tensor(out=m2[:], in0=mean[:], in1=mean[:], op=Alu.mult)
                    var = small.tile([128, NT], f32, tag="var")
                    nc.vector.tensor_scalar(out=var[:], in0=sums2[:], scalar1=1.0 / KH,
                                            scalar2=None, op0=Alu.mult)
                    nc.vector.tensor_tensor(out=var[:], in0=var[:], in1=m2[:], op=Alu.subtract)
                    # rstd = 1/sqrt(var + 1e-5), then fold in C_RED for range reduction
                    sd = small.tile([128, NT], f32, tag="sd")
                    nc.scalar.activation(out=sd[:], in_=var[:], func=Act.Sqrt,
                                         bias=eps[:], scale=1.0)
                    rstd = small.tile([128, NT], f32, tag="rstd")
                    nc.vector.reciprocal(out=rstd[:], in_=sd[:])
                    rstdc = small.tile([128, NT], f32, tag="rstdc")
                    nc.vector.tensor_scalar(out=rstdc[:], in0=rstd[:], scalar1=C_RED,
                                            scalar2=None, op0=Alu.mult)
                    # y = (x - mean) * rstdc   -> sin arg / 2pi
                    y = scratch.tile([128, NT, KH], f32, tag="y")
                    mean_v = bview(mean[:], [mean[:].ap[0], [1, NT], [0, KH]])
                    rstdc_v = bview(rstdc[:], [rstdc[:].ap[0], [1, NT], [0, KH]])
                    nc.vector.tensor_tensor(out=y[:], in0=x_ap, in1=mean_v, op=Alu.subtract)
                    nc.vector.tensor_tensor(out=y[:], in0=y[:], in1=rstdc_v, op=Alu.mult)
                    # f = y - round(y) via int32 round-cast
                    yi = scratch.tile([128, NT, KH], i32, tag="yi")
                    nc.vector.tensor_copy(out=yi[:], in_=y[:])
                    yif = scratch.tile([128, NT, KH], f32, tag="yif")
                    nc.vector.tensor_copy(out=yif[:], in_=yi[:])
                    nc.vector.tensor_tensor(out=y[:], in0=y[:], in1=yif[:], op=Alu.subtract)
                    # out = sin(2pi * f)
                    nc.scalar.activation(out=out_tile[:], in_=y[:], func=Act.Sin,
                                         scale=TWO_PI, bias=0.0)

                h1S = stage.tile([128, NT, KH], f32, tag=f"h1S_{net}")
                layernorm_sin(h1pre[:], h1S, f"l1{net}")

                # ---------- stage B: h2pre[:, t, :] = h1[:, t, :] @ w2 ----------
                h2pre_ps = psum_b.tile([128, NT, 64], f32, tag="h2pre", space="PSUM")
                for t in range(NT):
                    h1T_ps = psum_s.tile([KH, 128], f32, tag="h1T", space="PSUM")
                    nc.tensor.transpose(out=h1T_ps[:], in_=h1S[:, t, :], identity=ident[:])
                    h1T = small.tile([KH, 128], f32, tag="h1T_sb")
                    nc.scalar.activation(out=h1T[:], in_=h1T_ps[:], func=Act.Copy,
                                         bias=0.0, scale=1.0)
                    nc.tensor.matmul(out=h2pre_ps[:, t, 0:KH], lhsT=h1T[:], rhs=w2s[:, :],
                                     start=True, stop=True)

                h2pre_v = h2pre_ps[:, :, 0:KH]
                h2S = stage.tile([128, NT, KH], f16, tag=f"h2S_{net}")
                layernorm_sin(h2pre_v, h2S, f"l2{net}")

                # duplicate each h2 value (pairs along free dim) so the P-build
                # tensor_tensor reads aligned 16-bit pairs -> 2x DVE mode
                h2dupS = None
                if "h2dup" in parts:
                    h2dupS = stage.tile([128, NT, KH, 2], f16, tag=f"h2dup_{net}")
                    h2b = h2S[:]
                    h2_dup_view = bview(h2b, [h2b.ap[0], [KH, NT], [1, KH], [0, 2]])
                    nc.vector.tensor_copy(out=h2dupS[:], in_=h2_dup_view)

                # ---------- per-tile: P build, transpose, matmul chain ----------
                msgT_d = dram[f"msgT_{net}"]
                TGRP = 3  # e-tiles per DMA-transpose instruction
                for tp in range(0, NT, TGRP):
                    P = ptiles.tile([128, TGRP, KH, H], f16, tag="P")
                    PT = ptiles.tile([128, TGRP * 25, 128], f16, tag="PT")
                    if "pbuild" in parts:
                        for s in range(TGRP):
                            t = tp + s
                            emb_t = embS[:, t, :]
                            Ps = P[:, s, :, :]
                            if h2dupS is not None:
                                hd = h2dupS[:, t, :, :]
                                h2_v = bview(hd, [hd.ap[0], [2, KH], [0, 32], [1, 2]])
                                emb_v = bview(emb_t, [emb_t.ap[0], [0, KH], [2, 32], [1, 2]])
                                P_v = bview(Ps, [Ps.ap[0], [H, KH], [2, 32], [1, 2]])
                                nc.vector.tensor_tensor(out=P_v, in0=h2_v, in1=emb_v, op=Alu.mult)
                            else:
                                h2_t = h2S[:, t, :]
                                h2_v = bview(h2_t, [h2_t.ap[0], [1, KH], [0, H]])
                                emb_v = bview(emb_t, [emb_t.ap[0], [0, KH], [1, H]])
                                nc.vector.tensor_tensor(out=Ps, in0=h2_v, in1=emb_v, op=Alu.mult)

                    if "ptdma" in parts:
                        nc.sync.dma_start_transpose(
                            out=PT[:], in_=P[:].rearrange("p s j k -> p (s j k)"))

                    if "mm" in parts:
                        for s in range(TGRP):
                            t = tp + s
                            msg_ps = psum_m.tile([H, 128], f32, tag="msg", space="PSUM")
                            for c in range(25):
                                nc.tensor.matmul(out=msg_ps[:], lhsT=w3ps[:, c, :],
                                                 rhs=PT[:, s * 25 + c, :],
                                                 start=(c == 0), stop=(c == 24))
                            msg_sb = small.tile([H, 128], f32, tag="msg_sb")
                            nc.scalar.activation(out=msg_sb[:], in_=msg_ps[:], func=Act.Copy,
                                                 bias=0.0, scale=1.0)
                            meng = nc.gpsimd if "gmsg" in parts else nc.sync
                            meng.dma_start(out=msgT_d[:, t * 128:(t + 1) * 128], in_=msg_sb[:])

    nc.compile()
    return nc


_NC_CACHE = {}


def _get_nc(trivial_u, trivial_i, repeat=1, parts=("pbuild", "ptdma", "mm", "h2dup")):
    key = (trivial_u, trivial_i, repeat, tuple(parts))
    if key not in _NC_CACHE:
        _NC_CACHE[key] = _build_nc(trivial_u, trivial_i, repeat, parts)
    return _NC_CACHE[key]


def _trivial(p):
    names = ["w1", "b1", "g1", "be1", "w2", "b2", "g2", "be2", "w3", "b3"]
    d = dict(zip(names, p))
    return (np.all(d["b1"] == 0) and np.all(d["g1"] == 1) and np.all(d["be1"] == 0)
            and np.all(d["b2"] == 0) and np.all(d["g2"] == 1) and np.all(d["be2"] == 0)
            and np.all(d["b3"] == 0))


def prepare_in_maps(u_embedded, i_embedded, user_per_trans, item_per_trans, edges_t, u_t, i_t,
                    uw1, ub1, ug1, ube1, uw2, ub2, ug2, ube2, uw3, ub3,
                    iw1, ib1, ig1, ibe1, iw2, ib2, ig2, ibe2, iw3, ib3):
    u_embedded = np.asarray(u_embedded, np.float32)
    i_embedded = np.asarray(i_embedded, np.float32)
    user_idx = np.asarray(user_per_trans)
    item_idx = np.asarray(item_per_trans)
    edges_t = np.asarray(edges_t, np.float32)
    u_t = np.asarray(u_t, np.float32)
    i_t = np.asarray(i_t, np.float32)

    up = [np.asarray(x, np.float32) for x in (uw1, ub1, ug1, ube1, uw2, ub2, ug2, ube2, uw3, ub3)]
    ip = [np.asarray(x, np.float32) for x in (iw1, ib1, ig1, ibe1, iw2, ib2, ig2, ibe2, iw3, ib3)]
    triv_u, triv_i = _trivial(up), _trivial(ip)
    if not (triv_u and triv_i):
        raise NotImplementedError("non-trivial LN affine / bias params not supported")

    # host gather
    rel_u = (u_t[user_idx] - edges_t).astype(np.float32)       # [E]
    rel_i = (i_t[item_idx] - edges_t).astype(np.float32)
    gu = u_embedded[user_idx].astype(np.float16)               # [E, H]
    gi = i_embedded[item_idx].astype(np.float16)

    def w3perm(w3):
        # w3p[(j,k), h] = w3[j, h*64+k]; chunked then partition-major [128, 25, 64]
        w3r = w3.reshape(KH, H, H)                              # [j, h, k]
        w3p = np.ascontiguousarray(w3r.transpose(0, 2, 1)).reshape(25, 128, H)
        return np.ascontiguousarray(w3p.transpose(1, 0, 2)).astype(np.float16)

    consts = {}
    for net, p in (("u", up), ("i", ip)):
        consts[f"w1b_{net}"] = np.ascontiguousarray(np.tile(p[0][None, :], (128, 1)))
        consts[f"w2_{net}"] = np.ascontiguousarray(p[4])
        consts[f"w3p_{net}"] = w3perm(p[8])

    def shard(a, m):
        # pad core shard to EPT rows, then tile-major -> partition-major
        sl = a[m * EPC:(m + 1) * EPC]
        out = np.zeros((EPT,) + a.shape[1:], a.dtype)
        out[:EPC] = sl
        out = out.reshape((NT, 128) + a.shape[1:])
        return np.ascontiguousarray(np.moveaxis(out, 1, 0))    # [128, NT, ...]

    in_maps = []
    for m in range(NCORES):
        im = {
            "rel_u": shard(rel_u, m),
            "rel_i": shard(rel_i, m),
            "emb_u": shard(gu, m),
            "emb_i": shard(gi, m),
        }
        im.update(consts)
        in_maps.append(im)
    return in_maps, (triv_u, triv_i), (user_idx, item_idx)


def kernel(**inputs):
    from concourse.bass_utils import run_bass_kernel_spmd
    import os
    in_maps, (triv_u, triv_i), (user_idx, item_idx) = prepare_in_maps(**inputs)
    parts = tuple(os.environ.get("CK_PARTS", "pbuild,ptdma,mm,h2dup,gmsg").split(","))
    nc = _get_nc(triv_u, triv_i, 1, parts)
    trace = bool(os.environ.get("CK_TRACE"))
    res = run_bass_kernel_spmd(nc, in_maps, core_ids=list(range(NCORES)), trace=trace)
    global LAST_RESULT
    LAST_RESULT = res

    msg_u = np.concatenate([res.results[m]["msgT_u"].T[:EPC] for m in range(NCORES)], axis=0)
    msg_i = np.concatenate([res.results[m]["msgT_i"].T[:EPC] for m in range(NCORES)], axis=0)

    hLu = np.zeros((NU, H), np.float32)
    np.add.at(hLu, user_idx, msg_i)
    hLi = np.zeros((NI, H), np.float32)
    np.add.at(hLi, item_idx, msg_u)
    return (hLu, hLi)
